# revision 47
# baseline (speedup 1.0000x reference)
"""Trainium2 Bass kernel for nn_BaseDependentAttentionLayer (GNN edge attention).

Strategy (8 NeuronCores), v2 — edge-partition layout:
  - Shard nodes contiguously: core r owns origin nodes [1250r, 1250(r+1)).
  - Host sorts edges by origin; each core processes only its own origins'
    edges, so segment-softmax and scatter-add are core-local (no all-reduce).
  - LayerNorm split: center on device (x - mu), fold gain into weights,
    apply rstd as a per-row scale fused with the bias add (one DVE
    scalar_tensor_tensor per output) after the QKV matmuls.
  - k|v interleaved per node into one 2KB row; AllGather (bf16) then ONE
    dma_gather per edge-group fetches both (halves descriptor count).
  - Edge phase in edge-partition layout: q broadcast via one-hot matmul
    (host-precomputed stw), per-head score reduce via tensor_reduce over a
    [128, t, 8, 64] view, exp+per-head broadcast in one ACT op (stride-0
    AP), scatter-add and softmax denominator via one-hot matmul (host
    precomputed sT).
  - Softmax without max-subtraction (scores are O(1)); normalize after the
    scatter; MLP (W12 = W1@W2) + residual folded into each window epilogue.
"""

import sys

sys.path.insert(0, "/opt/trn_rl_repo")

import numpy as np
import ml_dtypes

bf16 = ml_dtypes.bfloat16

N, E, D, H = 10000, 160000, 512, 8
HD = D // H
SCALE = HD**-0.5
NCORES = 8
NPC = N // NCORES  # 1250 nodes/core
W = 10  # windows per core
WIN = 128  # origins per window
ET = 128  # edges per tile
EPS_LN = 1e-5
EPS_DEN = 1e-16
# destination-half split (window-aligned): per-core nodes [0,640) vs [640,1250)
HA = 640
HB = NPC - HA  # 610


def _host_prep(origin, dest, ew):
    """Sort edges by origin, bucket into (core, window, half, tile) slots.

    Within each window, edges whose dest lies in the first HA rows of its
    owner core come first (tiles [0,TA)), the rest after (tiles [TA,T)).
    didx holds the row index into kv_fullA / kv_fullB respectively.

    Returns (percore, TA, TB) with per-core dict:
      didx  [W, T*ET] int16  — gather row per edge slot (pad: 0)
      st    [128, W*T*ET] f32 — one-hot stw[o, slot] (origin-partition)
      sT    [128, W*T*128] f32 — one-hot sT[e, (w*T+t)*128 + o] (edge-partition)
      et    [128, W*T*8] f32  — edge weight ewt[e, (w*T+t)*8 + h] (pad: 0)
    """
    order = np.argsort(origin, kind="stable")
    o_s, d_s = origin[order], dest[order]
    core_of = o_s // NPC
    wloc = (o_s - core_of * NPC) // WIN
    in_a = (d_s % NPC) < HA
    countsA = np.zeros((NCORES, W), np.int64)
    countsB = np.zeros((NCORES, W), np.int64)
    for r in range(NCORES):
        cm = core_of == r
        for w in range(W):
            m = cm & (wloc == w)
            countsA[r, w] = int(np.sum(m & in_a))
            countsB[r, w] = int(np.sum(m & ~in_a))
    TA = max(1, int(np.ceil(countsA.max() / ET)))
    TB = max(1, int(np.ceil(countsB.max() / ET)))
    T = TA + TB
    rowA = (d_s // NPC) * HA + (d_s % NPC)
    rowB = (d_s // NPC) * HB + (d_s % NPC - HA)
    percore = []
    for r in range(NCORES):
        didx = np.zeros((W, T * ET), np.int16)
        oloc = np.full((W, T * ET), -1, np.int64)
        et = np.zeros((W, T * ET, H), np.float32)
        cm = core_of == r
        for w in range(W):
            m = cm & (wloc == w)
            ma = m & in_a
            mb = m & ~in_a
            ca, cb = int(ma.sum()), int(mb.sum())
            didx[w, :ca] = rowA[ma].astype(np.int16)
            oloc[w, :ca] = o_s[ma] - r * NPC - w * WIN
            et[w, :ca] = ew[order[ma]]
            b0 = TA * ET
            didx[w, b0:b0 + cb] = rowB[mb].astype(np.int16)
            oloc[w, b0:b0 + cb] = o_s[mb] - r * NPC - w * WIN
            et[w, b0:b0 + cb] = ew[order[mb]]
        # one-hots (vectorized)
        st = np.zeros((WIN, W * T * ET), np.float32)
        sT = np.zeros((ET, W * T * WIN), np.float32)
        vw, vi = np.nonzero(oloc >= 0)
        o = oloc[vw, vi]
        t = vi // ET
        e = vi % ET
        st[o, vw * T * ET + vi] = 1.0
        sT[e, (vw * T + t) * WIN + o] = 1.0
        etp = np.ascontiguousarray(et.reshape(W, T, ET, H).transpose(2, 0, 1, 3))
        percore.append(dict(
            didx=didx, st=st, sT=sT,
            et=etp.reshape(ET, W * T * H),
        ))
    return percore, TA, TB


def _gather_groups(TA, TB):
    """Split [0,TA) and [TA,TA+TB) tiles into even-sized groups (last of each
    half may be odd). Returns (t0, ng, half) triples."""
    gs = []
    for base, tn, half in ((0, TA, 0), (TA, TB, 1)):
        t = 0
        while t < tn:
            n = min(4, tn - t)
            if n == 3:
                n = 2  # keep groups even while possible
            gs.append((base + t, n, half))
            t += n
    return gs


def _wrap_idx(idx_flat):
    """int16 [n] -> wrapped [128, n/16] layout for dma_gather (idx i at
    [i%16, i//16], replicated over the 8 Q7 partition groups)."""
    w = idx_flat.reshape(-1, 16).T  # [16, n/16]
    return np.tile(w, (8, 1)).astype(np.int16)


def _build_program(TA, TB, mock_ag=False):
    import concourse.bass as bass
    import concourse.bacc as bacc
    import concourse.mybir as mybir
    import concourse.tile as tile

    dt = mybir.dt
    Alu = mybir.AluOpType
    Act = mybir.ActivationFunctionType

    T = TA + TB
    GROUPS = _gather_groups(TA, TB)

    nc = bacc.Bacc(
        "TRN2", target_bir_lowering=False, debug=False, num_devices=NCORES
    )

    # ---------------- I/O ----------------
    xs_t = nc.dram_tensor("xs", [W * 128, D], dt.float32, kind="ExternalInput")
    wq_t = nc.dram_tensor("wq", [128, 4, D], dt.bfloat16, kind="ExternalInput")
    wk_t = nc.dram_tensor("wk", [128, 4, D], dt.bfloat16, kind="ExternalInput")
    wv_t = nc.dram_tensor("wv", [128, 4, D], dt.bfloat16, kind="ExternalInput")
    w12_t = nc.dram_tensor("w12", [128, 4, D], dt.bfloat16, kind="ExternalInput")
    brep_t = nc.dram_tensor("brep", [128, 3, D], dt.bfloat16, kind="ExternalInput")
    b12_t = nc.dram_tensor("b12", [1, D], dt.bfloat16, kind="ExternalInput")
    ones_t = nc.dram_tensor("ones1", [1, 128], dt.bfloat16, kind="ExternalInput")
    ident_t = nc.dram_tensor("ident", [128, 128], dt.bfloat16, kind="ExternalInput")
    identf_t = nc.dram_tensor("identf", [128, 128], dt.float32, kind="ExternalInput")
    dkw_t = nc.dram_tensor("dkw", [128, W * T * ET // 16], dt.int16, kind="ExternalInput")
    st_t = nc.dram_tensor("st", [128, W * T * ET], dt.bfloat16, kind="ExternalInput")
    sT_t = nc.dram_tensor("sTt", [128, W * T * WIN], dt.bfloat16, kind="ExternalInput")
    ewt_t = nc.dram_tensor("ewt", [128, W * T * H], dt.bfloat16, kind="ExternalInput")
    out_t = nc.dram_tensor("out", [W * 128, D], dt.bfloat16, kind="ExternalOutput")

    with tile.TileContext(nc) as tc:
        with (
            tc.tile_pool(name="const", bufs=1) as cpool,
            tc.tile_pool(name="persist", bufs=1) as ppool,
            tc.tile_pool(name="dram", bufs=1, space="DRAM") as dpool,
        ):
            # constants
            wq = cpool.tile([128, 4, D], dt.bfloat16)
            wk = cpool.tile([128, 4, D], dt.bfloat16)
            wv = cpool.tile([128, 4, D], dt.bfloat16)
            w12 = cpool.tile([128, 4, D], dt.bfloat16)
            brep = cpool.tile([128, 3, D], dt.bfloat16)
            b12 = cpool.tile([1, D], dt.bfloat16)
            ones1 = cpool.tile([1, 128], dt.bfloat16)
            ident = cpool.tile([128, 128], dt.bfloat16)
            identf = cpool.tile([128, 128], dt.float32)
            dkw = cpool.tile([128, W * T * ET // 16], dt.int16)
            ewt = cpool.tile([128, W * T * H], dt.bfloat16)
            for tl, tn in [
                (wq, wq_t), (wk, wk_t), (wv, wv_t), (w12, w12_t),
                (brep, brep_t), (b12, b12_t), (ones1, ones_t),
                (ident, ident_t), (identf, identf_t),
                (dkw, dkw_t), (ewt, ewt_t),
            ]:
                nc.sync.dma_start(tl[:], tn.ap())

            # persistent activations
            q_sb = ppool.tile([128, W, D], dt.bfloat16)
            xall = ppool.tile([128, W, D], dt.float32)
            nc.sync.dma_start(
                xall[:], xs_t.ap().rearrange("(g p) d -> p g d", p=128)
            )

            # collective buffers (k|v interleaved per node: row = [k | v]),
            # split into two window-aligned halves so AG1 can start after
            # Phase-A window 4 and AG2 overlaps the first half of Phase B.
            kv_inA = dpool.tile([HA, 2 * D], dt.bfloat16)
            kv_inB = dpool.tile([HB, 2 * D], dt.bfloat16)
            kv_fullA = dpool.tile([NCORES * HA, 2 * D], dt.bfloat16,
                                  addr_space="Shared")
            kv_fullB = dpool.tile([NCORES * HB, 2 * D], dt.bfloat16,
                                  addr_space="Shared")

            # ---------------- PE warm-up ----------------
            # ~25 back-to-back matmuls at t=0 (overlapping the const DMAs)
            # flip the PE HAM clock gate from 1.2 to 2.4 GHz before Phase A's
            # real matmuls issue. Results are discarded.
            with tc.tile_pool(name="psW", bufs=1, space="PSUM") as psw:
                warm_ps = psw.tile([128, D], dt.float32, tag="warm")
                for _ in range(25):
                    nc.tensor.matmul(
                        warm_ps[:], ident[:], wq[:, 0, :], start=True, stop=True
                    )

            # ---------------- Phase A: LN + QKV ----------------
            with (
                tc.tile_pool(name="pA", bufs=4) as pa,
                tc.tile_pool(name="psA", bufs=2, space="PSUM") as psa,
            ):
                for g in range(W):
                    lo = g * 128
                    rows = max(0, min(128, NPC - lo))
                    xg = xall[:, g, :]
                    musum = pa.tile([128, 1], dt.float32, tag="musum")
                    nc.vector.tensor_reduce(musum[:], xg, mybir.AxisListType.X, Alu.add)
                    mu = pa.tile([128, 1], dt.float32, tag="mu")
                    nc.vector.tensor_scalar_mul(mu[:], musum[:], 1.0 / D)
                    xc = pa.tile([128, D], dt.float32, tag="xc")
                    nc.vector.tensor_scalar(xc[:], xg, mu[:], None, Alu.subtract)
                    sqd = pa.tile([128, D], dt.float32, tag="sqd")
                    vs = pa.tile([128, 1], dt.float32, tag="vs")
                    nc.scalar.activation(sqd[:], xc[:], Act.Square, accum_out=vs[:])
                    vr = pa.tile([128, 1], dt.float32, tag="vr")
                    nc.vector.tensor_scalar(vr[:], vs[:], 1.0 / D, EPS_LN, Alu.mult, Alu.add)
                    sd = pa.tile([128, 1], dt.float32, tag="sd")
                    nc.scalar.sqrt(sd[:], vr[:])
                    rstd = pa.tile([128, 1], dt.float32, tag="rstd")
                    nc.vector.reciprocal(rstd[:], sd[:])
                    zT_ps = psa.tile([128, 4, 128], dt.float32, tag="zT_ps")
                    for c in range(4):
                        nc.tensor.transpose(
                            zT_ps[:, c, :], xc[:, c * 128:(c + 1) * 128], identf[:]
                        )
                    zT = pa.tile([128, 4, 128], dt.bfloat16, tag="zT")
                    nc.scalar.copy(zT[:], zT_ps[:])
                    q_ps = psa.tile([128, D], dt.float32, tag="q_ps")
                    k_ps = psa.tile([128, D], dt.float32, tag="k_ps")
                    v_ps = psa.tile([128, D], dt.float32, tag="v_ps")
                    for c in range(4):
                        for ps, wt in [(k_ps, wk), (v_ps, wv), (q_ps, wq)]:
                            nc.tensor.matmul(
                                ps[:], zT[:, c, :], wt[:, c, :],
                                start=(c == 0), stop=(c == 3),
                            )
                    kvt = pa.tile([128, 2 * D], dt.bfloat16, tag="kvt")
                    nc.vector.scalar_tensor_tensor(
                        kvt[:, :D], k_ps[:], rstd[:], brep[:, 1, :], Alu.mult, Alu.add
                    )
                    nc.vector.scalar_tensor_tensor(
                        kvt[:, D:], v_ps[:], rstd[:], brep[:, 2, :], Alu.mult, Alu.add
                    )
                    nc.vector.scalar_tensor_tensor(
                        q_sb[:, g, :], q_ps[:], rstd[:], brep[:, 0, :], Alu.mult, Alu.add
                    )
                    if rows > 0:
                        if lo < HA:
                            nc.sync.dma_start(
                                kv_inA[lo:lo + rows, :], kvt[:rows, :]
                            )
                        else:
                            nc.sync.dma_start(
                                kv_inB[lo - HA:lo - HA + rows, :], kvt[:rows, :]
                            )
                    # AllGather half A as soon as windows 0-4 are done
                    if g == HA // 128 - 1:
                        if mock_ag:
                            nc.sync.dma_start(kv_fullA[0:HA, :], kv_inA[:])
                        else:
                            nc.gpsimd.collective_compute(
                                "AllGather",
                                Alu.bypass,
                                replica_groups=[list(range(NCORES))],
                                ins=[kv_inA.opt()],
                                outs=[kv_fullA.opt()],
                            )

            # ---------------- Phase A2: AllGather k|v half B ----------------
            if mock_ag:
                nc.sync.dma_start(kv_fullB[0:HB, :], kv_inB[:])
            else:
                nc.gpsimd.collective_compute(
                    "AllGather",
                    Alu.bypass,
                    replica_groups=[list(range(NCORES))],
                    ins=[kv_inB.opt()],
                    outs=[kv_fullB.opt()],
                )

            # ---------------- Phase B: edge loop + fused MLP ----------------
            with (
                tc.tile_pool(name="pB", bufs=2) as pb,
                tc.tile_pool(name="psQ", bufs=2, space="PSUM") as psq,
                tc.tile_pool(name="psAcc", bufs=1, space="PSUM") as psacc,
                tc.tile_pool(name="psE", bufs=1, space="PSUM") as pse,
            ):
                for w in range(W):
                    sTw = pb.tile([128, T, WIN], dt.bfloat16, tag="sTw")
                    nc.sync.dma_start(
                        sTw[:], sT_t.ap()[:, w * T * WIN:(w + 1) * T * WIN]
                    )
                    stww = pb.tile([128, T, ET], dt.bfloat16, tag="stww")
                    nc.sync.dma_start(
                        stww[:], st_t.ap()[:, w * T * ET:(w + 1) * T * ET]
                    )
                    kvgs = {}
                    for (t0, ng, half) in GROUPS:
                        kvG = pb.tile([128, ng, 2 * D], dt.bfloat16, tag=f"kv{t0}")
                        ni = ng * ET
                        c0 = (w * T + t0) * ET // 16
                        nc.gpsimd.dma_gather(
                            out_ap=kvG[:],
                            in_ap=(kv_fullB if half else kv_fullA)[:],
                            idxs_ap=dkw[:, c0:c0 + ni // 16],
                            num_idxs=ni, num_idxs_reg=ni, elem_size=2 * D,
                            single_packet=False,
                        )
                        kvgs[t0] = kvG

                    unnorm = psacc.tile([128, D], dt.float32, tag="unnorm")
                    den = psacc.tile([128, H], dt.float32, tag="den")

                    for (t0, ng, half) in GROUPS:
                        kvG = kvgs[t0]
                        j = 0
                        while j < ng:
                            np_ = min(2, ng - j)
                            tt = t0 + j  # first tile index in window
                            # Q broadcast to edge layout (PE)
                            qg_ps = psq.tile([128, 2, D], dt.float32, tag="qg_ps")
                            for i in range(np_):
                                nc.tensor.matmul(
                                    qg_ps[:, i, :],
                                    stww[:, tt + i, :], q_sb[:, w, :],
                                    start=True, stop=True,
                                )
                            qg_sb = pb.tile([128, 2, D], dt.bfloat16, tag="qg_sb")
                            nc.scalar.copy(qg_sb[:, :np_, :], qg_ps[:, :np_, :])
                            # kq = k ⊙ qg (DVE 2x)
                            kq = pb.tile([128, 2, D], dt.bfloat16, tag="kq")
                            nc.vector.tensor_tensor(
                                kq[:, :np_, :],
                                kvG[:, j:j + np_, :D],
                                qg_sb[:, :np_, :],
                                Alu.mult,
                            )
                            # per-head score reduce: two folded adds at 2x
                            # then a short 1x reduce (faster than one 64-wide
                            # 1x reduce)
                            kq5 = kq[:, :np_, :].rearrange(
                                "p a (h s d) -> p a h s d", h=H, s=2
                            )
                            f1 = pb.tile([128, 2, H, HD // 2], dt.bfloat16, tag="f1")
                            nc.vector.tensor_tensor(
                                f1[:, :np_], kq5[:, :, :, 0, :],
                                kq5[:, :, :, 1, :], Alu.add,
                            )
                            f15 = f1[:, :np_].rearrange(
                                "p a h (s d) -> p a h s d", s=2
                            )
                            f2 = pb.tile([128, 2, H, HD // 4], dt.bfloat16, tag="f2")
                            nc.vector.tensor_tensor(
                                f2[:, :np_], f15[:, :, :, 0, :],
                                f15[:, :, :, 1, :], Alu.add,
                            )
                            sc = pb.tile([128, 2, H], dt.float32, tag="sc")
                            nc.vector.tensor_reduce(
                                sc[:, :np_, :], f2[:, :np_],
                                mybir.AxisListType.X, Alu.add,
                            )
                            # ws = sc * ew (DVE, small)
                            ws = pb.tile([128, 2, H], dt.bfloat16, tag="ws")
                            nc.vector.tensor_tensor(
                                ws[:, :np_, :],
                                sc[:, :np_, :],
                                ewt[:, (w * T + tt) * H:(w * T + tt + np_) * H]
                                .rearrange("p (a h) -> p a h", h=H),
                                Alu.mult,
                            )
                            # exp + per-head broadcast (ACT, stride-0 input)
                            ewb = pb.tile([128, 2, D], dt.bfloat16, tag="ewb")
                            nc.scalar.activation(
                                ewb[:, :np_, :].rearrange(
                                    "p a (h d) -> p a h d", h=H
                                ),
                                ws[:, :np_, :].unsqueeze(-1)
                                .broadcast_to([128, np_, H, HD]),
                                Act.Exp,
                            )
                            # wv = v ⊙ ewb (DVE 2x)
                            wvt = pb.tile([128, 2, D], dt.bfloat16, tag="wvt")
                            nc.vector.tensor_tensor(
                                wvt[:, :np_, :],
                                kvG[:, j:j + np_, D:],
                                ewb[:, :np_, :],
                                Alu.mult,
                            )
                            # scatter-add + denominator (PE, accumulate)
                            for i in range(np_):
                                t = tt + i
                                nc.tensor.matmul(
                                    unnorm[:], sTw[:, t, :], wvt[:, i, :],
                                    start=(t == 0), stop=(t == T - 1),
                                )
                                nc.tensor.matmul(
                                    den[:], sTw[:, t, :], ewb[:, i, ::HD],
                                    start=(t == 0), stop=(t == T - 1),
                                )
                            j += np_

                    # ---- window epilogue: divide, MLP, residual ----
                    dene = pb.tile([128, H], dt.float32, tag="dene")
                    nc.vector.tensor_scalar(dene[:], den[:], EPS_DEN, None, Alu.add)
                    rec = pb.tile([128, H], dt.float32, tag="rec")
                    nc.vector.reciprocal(rec[:], dene[:])
                    vals = pb.tile([128, D], dt.bfloat16, tag="vals")
                    nc.vector.tensor_tensor(
                        vals[:].rearrange("p (h d) -> p h d", h=H),
                        unnorm[:].rearrange("p (h d) -> p h d", h=H),
                        rec[:].unsqueeze(-1).broadcast_to([128, H, HD]),
                        Alu.mult,
                    )
                    vT_ps = pse.tile([128, 4, 128], dt.bfloat16, tag="vT_ps")
                    for c in range(4):
                        nc.tensor.transpose(
                            vT_ps[:, c, :], vals[:, c * 128:(c + 1) * 128], ident[:]
                        )
                    vT = pb.tile([128, 4, 128], dt.bfloat16, tag="vT")
                    nc.scalar.copy(vT[:], vT_ps[:])
                    mlp_ps = pse.tile([128, D], dt.float32, tag="mlp")
                    for c in range(4):
                        nc.tensor.matmul(
                            mlp_ps[:], vT[:, c, :], w12[:, c, :],
                            start=(c == 0), stop=False,
                        )
                    nc.tensor.matmul(
                        mlp_ps[:], ones1[:], b12[:], start=False, stop=True
                    )
                    og = pb.tile([128, D], dt.bfloat16, tag="og")
                    nc.vector.tensor_tensor(og[:], mlp_ps[:], xall[:, w, :], Alu.add)
                    nc.sync.dma_start(out_t.ap()[w * 128:(w + 1) * 128, :], og[:])

    nc.compile()
    from concourse.bass_interp import get_hw_module

    nc.m = get_hw_module(nc.m)
    return nc


def kernel(x, edge_index, edge_weights, ln_g, ln_b, Wq, bq, Wk, bk, Wv, bv,
           W1, b1, W2, b2, _trace=False):
    x = np.asarray(x, np.float32)
    ei = np.asarray(edge_index)
    ew = np.asarray(edge_weights, np.float32)
    origin, dest = ei[0].astype(np.int64), ei[1].astype(np.int64)

    percore, TA, TB = _host_prep(origin, dest, ew)
    T = TA + TB

    # fold LN gain + attention scale into weights (host, fp32); rstd and the
    # (gain-folded) biases are applied on-device after the matmuls.
    ln_g = np.asarray(ln_g, np.float32)
    ln_b = np.asarray(ln_b, np.float32)
    Wq_f = (ln_g[:, None] * np.asarray(Wq, np.float32)) * SCALE
    bq_f = (ln_b @ np.asarray(Wq, np.float32)) * SCALE + np.asarray(bq, np.float32) * SCALE
    Wk_f = ln_g[:, None] * np.asarray(Wk, np.float32)
    bk_f = ln_b @ np.asarray(Wk, np.float32) + np.asarray(bk, np.float32)
    Wv_f = ln_g[:, None] * np.asarray(Wv, np.float32)
    bv_f = ln_b @ np.asarray(Wv, np.float32) + np.asarray(bv, np.float32)
    W12 = np.asarray(W1, np.float32) @ np.asarray(W2, np.float32)
    b12 = np.asarray(b1, np.float32) @ np.asarray(W2, np.float32) + np.asarray(b2, np.float32)

    def chunked(wm):  # [512, 512] -> [128, 4, 512]
        return np.ascontiguousarray(
            wm.reshape(4, 128, D).transpose(1, 0, 2)
        ).astype(bf16)

    brep = np.broadcast_to(
        np.stack([bq_f, bk_f, bv_f])[None], (128, 3, D)
    )

    common = dict(
        wq=chunked(Wq_f), wk=chunked(Wk_f), wv=chunked(Wv_f), w12=chunked(W12),
        brep=np.ascontiguousarray(brep).astype(bf16),
        b12=b12[None].astype(bf16),
        ones1=np.ones((1, 128), bf16),
        ident=np.eye(128, dtype=bf16),
        identf=np.eye(128, dtype=np.float32),
    )

    in_maps = []
    for r in range(NCORES):
        pc = percore[r]
        xs = np.zeros((W * 128, D), np.float32)
        xs[:NPC] = x[r * NPC:(r + 1) * NPC]
        # wrap indices per gather group
        didx = pc["didx"]  # [W, T*ET]
        dkw = np.zeros((128, W * T * ET // 16), np.int16)
        for w in range(W):
            for (t0, ng, half) in _gather_groups(TA, TB):
                ni = ng * ET
                c0 = (w * T + t0) * ET // 16
                dkw[:, c0:c0 + ni // 16] = _wrap_idx(
                    didx[w, t0 * ET:t0 * ET + ni]
                )
        in_maps.append(dict(
            xs=xs,
            dkw=dkw,
            st=np.ascontiguousarray(pc["st"]).astype(bf16),
            sTt=np.ascontiguousarray(pc["sT"]).astype(bf16),
            ewt=np.ascontiguousarray(pc["et"]).astype(bf16),
            **common,
        ))

    nc = _build_program(TA, TB)
    from concourse import bass_utils

    res = bass_utils.run_bass_kernel_spmd(
        nc, in_maps, core_ids=list(range(NCORES)),
        trace=bool(_trace),
        tmpdir=("/root/problem/work/trace" if _trace else None),
    )
    out = np.concatenate(
        [res.results[r]["out"][:NPC] for r in range(NCORES)], axis=0
    )
    kernel.last_result = res
    if _trace and res.exec_time_ns is not None:
        kernel.exec_time_ns = res.exec_time_ns
    return out.astype(np.float32)


# revision 53
# speedup vs baseline: 1.1253x; 1.1253x over previous
"""Trainium2 Bass kernel for nn_BaseDependentAttentionLayer (GNN edge attention).

Strategy (8 NeuronCores), v2 — edge-partition layout:
  - Shard nodes contiguously: core r owns origin nodes [1250r, 1250(r+1)).
  - Host sorts edges by origin; each core processes only its own origins'
    edges, so segment-softmax and scatter-add are core-local (no all-reduce).
  - LayerNorm split: center on device (x - mu), fold gain into weights,
    apply rstd as a per-row scale fused with the bias add (one DVE
    scalar_tensor_tensor per output) after the QKV matmuls.
  - k|v interleaved per node into one 2KB row; AllGather (bf16) then ONE
    dma_gather per edge-group fetches both (halves descriptor count).
  - Edge phase in edge-partition layout: q broadcast via one-hot matmul
    (host-precomputed stw), per-head score reduce via tensor_reduce over a
    [128, t, 8, 64] view, exp+per-head broadcast in one ACT op (stride-0
    AP), scatter-add and softmax denominator via one-hot matmul (host
    precomputed sT).
  - Softmax without max-subtraction (scores are O(1)); normalize after the
    scatter; MLP (W12 = W1@W2) + residual folded into each window epilogue.
"""

import sys

sys.path.insert(0, "/opt/trn_rl_repo")

import numpy as np
import ml_dtypes

bf16 = ml_dtypes.bfloat16

N, E, D, H = 10000, 160000, 512, 8
HD = D // H
SCALE = HD**-0.5
NCORES = 8
NPC = N // NCORES  # 1250 nodes/core
W = 10  # windows per core
WIN = 128  # origins per window
ET = 128  # edges per tile
EPS_LN = 1e-5
EPS_DEN = 1e-16
# destination-half split (window-aligned): per-core nodes [0,640) vs [640,1250)
HA = 640
HB = NPC - HA  # 610


def _host_prep(origin, dest, ew):
    """Sort edges by origin, bucket into (core, window, half, tile) slots.

    Within each window, edges whose dest lies in the first HA rows of its
    owner core come first (tiles [0,TA)), the rest after (tiles [TA,T)).
    didx holds the row index into kv_fullA / kv_fullB respectively.

    Returns (percore, TA, TB) with per-core dict:
      didx  [W, T*ET] int16  — gather row per edge slot (pad: 0)
      st    [128, W*T*ET] f32 — one-hot stw[o, slot] (origin-partition)
      sT    [128, W*T*128] f32 — one-hot sT[e, (w*T+t)*128 + o] (edge-partition)
      et    [128, W*T*8] f32  — edge weight ewt[e, (w*T+t)*8 + h] (pad: 0)
    """
    order = np.argsort(origin, kind="stable")
    o_s, d_s = origin[order], dest[order]
    core_of = o_s // NPC
    wloc = (o_s - core_of * NPC) // WIN
    in_a = (d_s % NPC) < HA
    countsA = np.zeros((NCORES, W), np.int64)
    countsB = np.zeros((NCORES, W), np.int64)
    for r in range(NCORES):
        cm = core_of == r
        for w in range(W):
            m = cm & (wloc == w)
            countsA[r, w] = int(np.sum(m & in_a))
            countsB[r, w] = int(np.sum(m & ~in_a))
    TA = max(1, int(np.ceil(countsA.max() / ET)))
    TB = max(1, int(np.ceil(countsB.max() / ET)))
    T = TA + TB
    rowA = (d_s // NPC) * HA + (d_s % NPC)
    rowB = (d_s // NPC) * HB + (d_s % NPC - HA)
    percore = []
    for r in range(NCORES):
        didx = np.zeros((W, T * ET), np.int16)
        oloc = np.full((W, T * ET), -1, np.int64)
        et = np.zeros((W, T * ET, H), np.float32)
        cm = core_of == r
        for w in range(W):
            m = cm & (wloc == w)
            ma = m & in_a
            mb = m & ~in_a
            ca, cb = int(ma.sum()), int(mb.sum())
            didx[w, :ca] = rowA[ma].astype(np.int16)
            oloc[w, :ca] = o_s[ma] - r * NPC - w * WIN
            et[w, :ca] = ew[order[ma]]
            b0 = TA * ET
            didx[w, b0:b0 + cb] = rowB[mb].astype(np.int16)
            oloc[w, b0:b0 + cb] = o_s[mb] - r * NPC - w * WIN
            et[w, b0:b0 + cb] = ew[order[mb]]
        # one-hots (vectorized)
        st = np.zeros((WIN, W * T * ET), np.float32)
        sT = np.zeros((ET, W * T * WIN), np.float32)
        vw, vi = np.nonzero(oloc >= 0)
        o = oloc[vw, vi]
        t = vi // ET
        e = vi % ET
        st[o, vw * T * ET + vi] = 1.0
        sT[e, (vw * T + t) * WIN + o] = 1.0
        etp = np.ascontiguousarray(et.reshape(W, T, ET, H).transpose(2, 0, 1, 3))
        percore.append(dict(
            didx=didx, st=st, sT=sT,
            et=etp.reshape(ET, W * T * H),
        ))
    return percore, TA, TB


def _gather_groups(TA, TB):
    """Split [0,TA) and [TA,TA+TB) tiles into even-sized groups (last of each
    half may be odd). Returns (t0, ng, half) triples."""
    gs = []
    for base, tn, half in ((0, TA, 0), (TA, TB, 1)):
        t = 0
        while t < tn:
            n = min(4, tn - t)
            if n == 3:
                n = 2  # keep groups even while possible
            gs.append((base + t, n, half))
            t += n
    return gs


def _wrap_idx(idx_flat):
    """int16 [n] -> wrapped [128, n/16] layout for dma_gather (idx i at
    [i%16, i//16], replicated over the 8 Q7 partition groups)."""
    w = idx_flat.reshape(-1, 16).T  # [16, n/16]
    return np.tile(w, (8, 1)).astype(np.int16)


def _build_program(TA, TB, mock_ag=False):
    import concourse.bass as bass
    import concourse.bacc as bacc
    import concourse.mybir as mybir
    import concourse.tile as tile

    dt = mybir.dt
    Alu = mybir.AluOpType
    Act = mybir.ActivationFunctionType

    T = TA + TB
    GROUPS = _gather_groups(TA, TB)

    nc = bacc.Bacc(
        "TRN2", target_bir_lowering=False, debug=False, num_devices=NCORES
    )

    # ---------------- I/O ----------------
    xs_t = nc.dram_tensor("xs", [W * 128, D], dt.float32, kind="ExternalInput")
    wq_t = nc.dram_tensor("wq", [128, 4, D], dt.bfloat16, kind="ExternalInput")
    wk_t = nc.dram_tensor("wk", [128, 4, D], dt.bfloat16, kind="ExternalInput")
    wv_t = nc.dram_tensor("wv", [128, 4, D], dt.bfloat16, kind="ExternalInput")
    w12_t = nc.dram_tensor("w12", [128, 4, D], dt.bfloat16, kind="ExternalInput")
    brep_t = nc.dram_tensor("brep", [128, 3, D], dt.bfloat16, kind="ExternalInput")
    b12_t = nc.dram_tensor("b12", [1, D], dt.bfloat16, kind="ExternalInput")
    ones_t = nc.dram_tensor("ones1", [1, 128], dt.bfloat16, kind="ExternalInput")
    ident_t = nc.dram_tensor("ident", [128, 128], dt.bfloat16, kind="ExternalInput")
    identf_t = nc.dram_tensor("identf", [128, 128], dt.float32, kind="ExternalInput")
    dkw_t = nc.dram_tensor("dkw", [128, W * T * ET // 16], dt.int16, kind="ExternalInput")
    st_t = nc.dram_tensor("st", [128, W * T * ET], dt.bfloat16, kind="ExternalInput")
    sT_t = nc.dram_tensor("sTt", [128, W * T * WIN], dt.bfloat16, kind="ExternalInput")
    ewt_t = nc.dram_tensor("ewt", [128, W * T * H], dt.bfloat16, kind="ExternalInput")
    out_t = nc.dram_tensor("out", [W * 128, D], dt.bfloat16, kind="ExternalOutput")

    with tile.TileContext(nc) as tc:
        with (
            tc.tile_pool(name="const", bufs=1) as cpool,
            tc.tile_pool(name="persist", bufs=1) as ppool,
            tc.tile_pool(name="dram", bufs=1, space="DRAM") as dpool,
        ):
            # constants
            wq = cpool.tile([128, 4, D], dt.bfloat16)
            wk = cpool.tile([128, 4, D], dt.bfloat16)
            wv = cpool.tile([128, 4, D], dt.bfloat16)
            w12 = cpool.tile([128, 4, D], dt.bfloat16)
            brep = cpool.tile([128, 3, D], dt.bfloat16)
            b12 = cpool.tile([1, D], dt.bfloat16)
            ones1 = cpool.tile([1, 128], dt.bfloat16)
            ident = cpool.tile([128, 128], dt.bfloat16)
            identf = cpool.tile([128, 128], dt.float32)
            dkw = cpool.tile([128, W * T * ET // 16], dt.int16)
            ewt = cpool.tile([128, W * T * H], dt.bfloat16)
            for tl, tn in [
                (wq, wq_t), (wk, wk_t), (wv, wv_t), (w12, w12_t),
                (brep, brep_t), (b12, b12_t), (ones1, ones_t),
                (ident, ident_t), (identf, identf_t),
                (dkw, dkw_t), (ewt, ewt_t),
            ]:
                nc.sync.dma_start(tl[:], tn.ap())

            # persistent activations
            q_sb = ppool.tile([128, W, D], dt.bfloat16)
            xall = ppool.tile([128, W, D], dt.float32)
            nc.sync.dma_start(
                xall[:], xs_t.ap().rearrange("(g p) d -> p g d", p=128)
            )

            # collective buffers (k|v interleaved per node: row = [k | v]),
            # split into two window-aligned halves so AG1 can start after
            # Phase-A window 4 and AG2 overlaps the first half of Phase B.
            kv_inA = dpool.tile([HA, 2 * D], dt.bfloat16)
            kv_inB = dpool.tile([HB, 2 * D], dt.bfloat16)
            kv_fullA = dpool.tile([NCORES * HA, 2 * D], dt.bfloat16,
                                  addr_space="Shared")
            kv_fullB = dpool.tile([NCORES * HB, 2 * D], dt.bfloat16,
                                  addr_space="Shared")

            # ---------------- Phase A: LN + QKV ----------------
            with (
                tc.tile_pool(name="pA", bufs=4) as pa,
                tc.tile_pool(name="psA", bufs=2, space="PSUM") as psa,
            ):
                for g in range(W):
                    lo = g * 128
                    rows = max(0, min(128, NPC - lo))
                    xg = xall[:, g, :]
                    musum = pa.tile([128, 1], dt.float32, tag="musum")
                    nc.vector.tensor_reduce(musum[:], xg, mybir.AxisListType.X, Alu.add)
                    mu = pa.tile([128, 1], dt.float32, tag="mu")
                    nc.vector.tensor_scalar_mul(mu[:], musum[:], 1.0 / D)
                    xc = pa.tile([128, D], dt.float32, tag="xc")
                    nc.vector.tensor_scalar(xc[:], xg, mu[:], None, Alu.subtract)
                    sqd = pa.tile([128, D], dt.float32, tag="sqd")
                    vs = pa.tile([128, 1], dt.float32, tag="vs")
                    nc.scalar.activation(sqd[:], xc[:], Act.Square, accum_out=vs[:])
                    vr = pa.tile([128, 1], dt.float32, tag="vr")
                    nc.vector.tensor_scalar(vr[:], vs[:], 1.0 / D, EPS_LN, Alu.mult, Alu.add)
                    sd = pa.tile([128, 1], dt.float32, tag="sd")
                    nc.scalar.sqrt(sd[:], vr[:])
                    rstd = pa.tile([128, 1], dt.float32, tag="rstd")
                    nc.vector.reciprocal(rstd[:], sd[:])
                    zT_ps = psa.tile([128, 4, 128], dt.float32, tag="zT_ps")
                    for c in range(4):
                        nc.tensor.transpose(
                            zT_ps[:, c, :], xc[:, c * 128:(c + 1) * 128], identf[:]
                        )
                    zT = pa.tile([128, 4, 128], dt.bfloat16, tag="zT")
                    nc.scalar.copy(zT[:], zT_ps[:])
                    q_ps = psa.tile([128, D], dt.float32, tag="q_ps")
                    k_ps = psa.tile([128, D], dt.float32, tag="k_ps")
                    v_ps = psa.tile([128, D], dt.float32, tag="v_ps")
                    for c in range(4):
                        for ps, wt in [(k_ps, wk), (v_ps, wv), (q_ps, wq)]:
                            nc.tensor.matmul(
                                ps[:], zT[:, c, :], wt[:, c, :],
                                start=(c == 0), stop=(c == 3),
                            )
                    kvt = pa.tile([128, 2 * D], dt.bfloat16, tag="kvt")
                    nc.vector.scalar_tensor_tensor(
                        kvt[:, :D], k_ps[:], rstd[:], brep[:, 1, :], Alu.mult, Alu.add
                    )
                    # v half stored h-minor (col j*8+h = head h, dim j) so the
                    # per-head exp broadcast in Phase B has contiguous stride
                    nc.vector.scalar_tensor_tensor(
                        kvt[:, D:].rearrange("p (j h) -> p h j", h=H),
                        v_ps[:].rearrange("p (h j) -> p h j", h=H),
                        rstd[:],
                        brep[:, 2, :].rearrange("p (h j) -> p h j", h=H),
                        Alu.mult, Alu.add,
                    )
                    nc.vector.scalar_tensor_tensor(
                        q_sb[:, g, :], q_ps[:], rstd[:], brep[:, 0, :], Alu.mult, Alu.add
                    )
                    if rows > 0:
                        if lo < HA:
                            nc.sync.dma_start(
                                kv_inA[lo:lo + rows, :], kvt[:rows, :]
                            )
                        else:
                            nc.sync.dma_start(
                                kv_inB[lo - HA:lo - HA + rows, :], kvt[:rows, :]
                            )
                    # AllGather half A as soon as windows 0-4 are done
                    if g == HA // 128 - 1:
                        if mock_ag:
                            nc.sync.dma_start(kv_fullA[0:HA, :], kv_inA[:])
                        else:
                            nc.gpsimd.collective_compute(
                                "AllGather",
                                Alu.bypass,
                                replica_groups=[list(range(NCORES))],
                                ins=[kv_inA.opt()],
                                outs=[kv_fullA.opt()],
                            )

            # ---------------- Phase A2: AllGather k|v half B ----------------
            if mock_ag:
                nc.sync.dma_start(kv_fullB[0:HB, :], kv_inB[:])
            else:
                nc.gpsimd.collective_compute(
                    "AllGather",
                    Alu.bypass,
                    replica_groups=[list(range(NCORES))],
                    ins=[kv_inB.opt()],
                    outs=[kv_fullB.opt()],
                )

            # ---------------- Phase B: edge loop + fused MLP ----------------
            with (
                tc.tile_pool(name="pB", bufs=2) as pb,
                tc.tile_pool(name="psQ", bufs=2, space="PSUM") as psq,
                tc.tile_pool(name="psAcc", bufs=1, space="PSUM") as psacc,
                tc.tile_pool(name="psE", bufs=1, space="PSUM") as pse,
            ):
                for w in range(W):
                    sTw = pb.tile([128, T, WIN], dt.bfloat16, tag="sTw")
                    nc.sync.dma_start(
                        sTw[:], sT_t.ap()[:, w * T * WIN:(w + 1) * T * WIN]
                    )
                    stww = pb.tile([128, T, ET], dt.bfloat16, tag="stww")
                    nc.sync.dma_start(
                        stww[:], st_t.ap()[:, w * T * ET:(w + 1) * T * ET]
                    )
                    kvgs = {}
                    for (t0, ng, half) in GROUPS:
                        kvG = pb.tile([128, ng, 2 * D], dt.bfloat16, tag=f"kv{t0}")
                        ni = ng * ET
                        c0 = (w * T + t0) * ET // 16
                        nc.gpsimd.dma_gather(
                            out_ap=kvG[:],
                            in_ap=(kv_fullB if half else kv_fullA)[:],
                            idxs_ap=dkw[:, c0:c0 + ni // 16],
                            num_idxs=ni, num_idxs_reg=ni, elem_size=2 * D,
                            single_packet=False,
                        )
                        kvgs[t0] = kvG

                    unnorm = psacc.tile([128, D], dt.float32, tag="unnorm")
                    den = psacc.tile([128, H], dt.float32, tag="den")

                    for (t0, ng, half) in GROUPS:
                        kvG = kvgs[t0]
                        j = 0
                        while j < ng:
                            np_ = min(2, ng - j)
                            tt = t0 + j  # first tile index in window
                            # Q broadcast to edge layout (PE)
                            qg_ps = psq.tile([128, 2, D], dt.float32, tag="qg_ps")
                            for i in range(np_):
                                nc.tensor.matmul(
                                    qg_ps[:, i, :],
                                    stww[:, tt + i, :], q_sb[:, w, :],
                                    start=True, stop=True,
                                )
                            qg_sb = pb.tile([128, 2, D], dt.bfloat16, tag="qg_sb")
                            nc.scalar.copy(qg_sb[:, :np_, :], qg_ps[:, :np_, :])
                            # kq = k ⊙ qg (DVE 2x)
                            kq = pb.tile([128, 2, D], dt.bfloat16, tag="kq")
                            nc.vector.tensor_tensor(
                                kq[:, :np_, :],
                                kvG[:, j:j + np_, :D],
                                qg_sb[:, :np_, :],
                                Alu.mult,
                            )
                            # per-head score reduce: two folded adds at 2x
                            # then a short 1x reduce (faster than one 64-wide
                            # 1x reduce)
                            kq5 = kq[:, :np_, :].rearrange(
                                "p a (h s d) -> p a h s d", h=H, s=2
                            )
                            f1 = pb.tile([128, 2, H, HD // 2], dt.bfloat16, tag="f1")
                            nc.vector.tensor_tensor(
                                f1[:, :np_], kq5[:, :, :, 0, :],
                                kq5[:, :, :, 1, :], Alu.add,
                            )
                            f15 = f1[:, :np_].rearrange(
                                "p a h (s d) -> p a h s d", s=2
                            )
                            f2 = pb.tile([128, 2, H, HD // 4], dt.bfloat16, tag="f2")
                            nc.vector.tensor_tensor(
                                f2[:, :np_], f15[:, :, :, 0, :],
                                f15[:, :, :, 1, :], Alu.add,
                            )
                            sc = pb.tile([128, 2, H], dt.float32, tag="sc")
                            nc.vector.tensor_reduce(
                                sc[:, :np_, :], f2[:, :np_],
                                mybir.AxisListType.X, Alu.add,
                            )
                            # ws = sc * ew (DVE, small)
                            ws = pb.tile([128, 2, H], dt.bfloat16, tag="ws")
                            nc.vector.tensor_tensor(
                                ws[:, :np_, :],
                                sc[:, :np_, :],
                                ewt[:, (w * T + tt) * H:(w * T + tt + np_) * H]
                                .rearrange("p (a h) -> p a h", h=H),
                                Alu.mult,
                            )
                            # exp + per-head broadcast (ACT); v is h-minor so
                            # the broadcast input has contiguous innermost h
                            ewb = pb.tile([128, 2, D], dt.bfloat16, tag="ewb")
                            nc.scalar.activation(
                                ewb[:, :np_, :].rearrange(
                                    "p a (j h) -> p a j h", h=H
                                ),
                                ws[:, :np_, :].unsqueeze(2)
                                .broadcast_to([128, np_, HD, H]),
                                Act.Exp,
                            )
                            # wv = v ⊙ ewb (DVE 2x)
                            wvt = pb.tile([128, 2, D], dt.bfloat16, tag="wvt")
                            nc.vector.tensor_tensor(
                                wvt[:, :np_, :],
                                kvG[:, j:j + np_, D:],
                                ewb[:, :np_, :],
                                Alu.mult,
                            )
                            # scatter-add + denominator (PE, accumulate)
                            for i in range(np_):
                                t = tt + i
                                nc.tensor.matmul(
                                    unnorm[:], sTw[:, t, :], wvt[:, i, :],
                                    start=(t == 0), stop=(t == T - 1),
                                )
                                nc.tensor.matmul(
                                    den[:], sTw[:, t, :], ewb[:, i, 0:H],
                                    start=(t == 0), stop=(t == T - 1),
                                )
                            j += np_

                    # ---- window epilogue: divide, MLP, residual ----
                    dene = pb.tile([128, H], dt.float32, tag="dene")
                    nc.vector.tensor_scalar(dene[:], den[:], EPS_DEN, None, Alu.add)
                    rec = pb.tile([128, H], dt.float32, tag="rec")
                    nc.vector.reciprocal(rec[:], dene[:])
                    vals = pb.tile([128, D], dt.bfloat16, tag="vals")
                    nc.vector.tensor_tensor(
                        vals[:].rearrange("p (j h) -> p j h", h=H),
                        unnorm[:].rearrange("p (j h) -> p j h", h=H),
                        rec[:].unsqueeze(1).broadcast_to([128, HD, H]),
                        Alu.mult,
                    )
                    vT_ps = pse.tile([128, 4, 128], dt.bfloat16, tag="vT_ps")
                    for c in range(4):
                        nc.tensor.transpose(
                            vT_ps[:, c, :], vals[:, c * 128:(c + 1) * 128], ident[:]
                        )
                    vT = pb.tile([128, 4, 128], dt.bfloat16, tag="vT")
                    nc.scalar.copy(vT[:], vT_ps[:])
                    mlp_ps = pse.tile([128, D], dt.float32, tag="mlp")
                    for c in range(4):
                        nc.tensor.matmul(
                            mlp_ps[:], vT[:, c, :], w12[:, c, :],
                            start=(c == 0), stop=False,
                        )
                    nc.tensor.matmul(
                        mlp_ps[:], ones1[:], b12[:], start=False, stop=True
                    )
                    og = pb.tile([128, D], dt.bfloat16, tag="og")
                    nc.vector.tensor_tensor(og[:], mlp_ps[:], xall[:, w, :], Alu.add)
                    nc.sync.dma_start(out_t.ap()[w * 128:(w + 1) * 128, :], og[:])

    nc.compile()
    from concourse.bass_interp import get_hw_module

    nc.m = get_hw_module(nc.m)
    return nc


def kernel(x, edge_index, edge_weights, ln_g, ln_b, Wq, bq, Wk, bk, Wv, bv,
           W1, b1, W2, b2, _trace=False):
    x = np.asarray(x, np.float32)
    ei = np.asarray(edge_index)
    ew = np.asarray(edge_weights, np.float32)
    origin, dest = ei[0].astype(np.int64), ei[1].astype(np.int64)

    percore, TA, TB = _host_prep(origin, dest, ew)
    T = TA + TB

    # fold LN gain + attention scale into weights (host, fp32); rstd and the
    # (gain-folded) biases are applied on-device after the matmuls.
    ln_g = np.asarray(ln_g, np.float32)
    ln_b = np.asarray(ln_b, np.float32)
    Wq_f = (ln_g[:, None] * np.asarray(Wq, np.float32)) * SCALE
    bq_f = (ln_b @ np.asarray(Wq, np.float32)) * SCALE + np.asarray(bq, np.float32) * SCALE
    Wk_f = ln_g[:, None] * np.asarray(Wk, np.float32)
    bk_f = ln_b @ np.asarray(Wk, np.float32) + np.asarray(bk, np.float32)
    Wv_f = ln_g[:, None] * np.asarray(Wv, np.float32)
    bv_f = ln_b @ np.asarray(Wv, np.float32) + np.asarray(bv, np.float32)
    W12 = np.asarray(W1, np.float32) @ np.asarray(W2, np.float32)
    # values arrive h-minor (col j*8+h = original col h*64+j): permute W12 rows
    cidx = np.arange(D)
    W12 = W12[(cidx % H) * HD + cidx // H]
    b12 = np.asarray(b1, np.float32) @ np.asarray(W2, np.float32) + np.asarray(b2, np.float32)

    def chunked(wm):  # [512, 512] -> [128, 4, 512]
        return np.ascontiguousarray(
            wm.reshape(4, 128, D).transpose(1, 0, 2)
        ).astype(bf16)

    brep = np.broadcast_to(
        np.stack([bq_f, bk_f, bv_f])[None], (128, 3, D)
    )

    common = dict(
        wq=chunked(Wq_f), wk=chunked(Wk_f), wv=chunked(Wv_f), w12=chunked(W12),
        brep=np.ascontiguousarray(brep).astype(bf16),
        b12=b12[None].astype(bf16),
        ones1=np.ones((1, 128), bf16),
        ident=np.eye(128, dtype=bf16),
        identf=np.eye(128, dtype=np.float32),
    )

    in_maps = []
    for r in range(NCORES):
        pc = percore[r]
        xs = np.zeros((W * 128, D), np.float32)
        xs[:NPC] = x[r * NPC:(r + 1) * NPC]
        # wrap indices per gather group
        didx = pc["didx"]  # [W, T*ET]
        dkw = np.zeros((128, W * T * ET // 16), np.int16)
        for w in range(W):
            for (t0, ng, half) in _gather_groups(TA, TB):
                ni = ng * ET
                c0 = (w * T + t0) * ET // 16
                dkw[:, c0:c0 + ni // 16] = _wrap_idx(
                    didx[w, t0 * ET:t0 * ET + ni]
                )
        in_maps.append(dict(
            xs=xs,
            dkw=dkw,
            st=np.ascontiguousarray(pc["st"]).astype(bf16),
            sTt=np.ascontiguousarray(pc["sT"]).astype(bf16),
            ewt=np.ascontiguousarray(pc["et"]).astype(bf16),
            **common,
        ))

    nc = _build_program(TA, TB)
    from concourse import bass_utils

    res = bass_utils.run_bass_kernel_spmd(
        nc, in_maps, core_ids=list(range(NCORES)),
        trace=bool(_trace),
        tmpdir=("/root/problem/work/trace" if _trace else None),
    )
    out = np.concatenate(
        [res.results[r]["out"][:NPC] for r in range(NCORES)], axis=0
    )
    kernel.last_result = res
    if _trace and res.exec_time_ns is not None:
        kernel.exec_time_ns = res.exec_time_ns
    return out.astype(np.float32)
